# revision 1
# baseline (speedup 1.0000x reference)
"""Trainium2 Bass kernel for nn_DotAttention (B=4, Tq=Tv=2048, D=1024, 16 heads).

Sharding: core c -> (batch b = c//2, head-group hg = c%2 of 8 heads).
Each core computes q/k/v projections for its 512 att-dim slice, masked
softmax attention in transposed-energy layout, and a partial final
projection with its 512-row slice of Wf. Host sums the two partials per
batch and adds the bias constant (bv @ Wf + bf, exact because attention
weights sum to 1).

Layouts (SBUF is [128 partitions, free]):
  qT/kT  [128, 4, T]   partition+chunk = att-dim slice c, free = time
  v      [128, NJ, 520] partition = Tv tile, per head 65 cols (64 v + ones)
  energy^T in PSUM [128(Tv), 2*512] both heads of a pair side by side
  ctx^T  in PSUM [65, 512] per head; row 64 = softmax denominator (ones col)

All matmuls run in float32r (TF32-like, full PE rate at N>=256).
The program is specialized on NJ = ceil(max(value_lens)/128): fully
masked Tv chunks beyond that are skipped; per-core masking is handled by
a per-partition additive bias (-1e30) on the exp activation.
"""

import sys

sys.path.insert(0, "/opt/trn_rl_repo")

import numpy as np

import concourse.bacc as bacc
import concourse.tile as tile
import concourse.mybir as mybir
from concourse.bass_utils import run_bass_kernel_spmd

F32 = mybir.dt.float32
F32R = mybir.dt.float32r
BF16 = mybir.dt.bfloat16
F16 = mybir.dt.float16
MMDT = F32R
AF = mybir.ActivationFunctionType

B, T, D, ATT = 4, 2048, 1024, 1024
NH, DH = 16, 64
HPC = 8  # heads per core
CD = 512  # att-dim slice per core
NCORES = 8
LARGE = 1e30
SW = 512  # time-span width per streamed input chunk

_cache = {}


def build_nc(NJ, phases="ABC", loop_n=1, mmdt=None, splice=True,
             ebufs=2, pcybufs=4, exbufs=3, ehalf=False, pebc=False,
             qkf16=False):
    global MMDT
    if mmdt is not None:
        MMDT = mmdt
    key = (NJ, phases, loop_n, str(MMDT), splice, ebufs, pcybufs, exbufs, ehalf, pebc, qkf16)
    QKDT = F16 if qkf16 else MMDT
    if key in _cache:
        return _cache[key]
    NSV = (NJ * 128 + SW - 1) // SW  # spans of Tv needed for k/v
    TV = NSV * SW  # padded Tv extent materialized for kT
    nc = bacc.Bacc("TRN2", target_bir_lowering=False, debug=False, num_devices=NCORES)

    xq_d = nc.dram_tensor("xq", [D, T], MMDT, kind="ExternalInput")  # query[b].T
    xv_d = nc.dram_tensor("xv", [D, T], MMDT, kind="ExternalInput")  # value[b].T
    wq_d = nc.dram_tensor("wq", [D, CD], MMDT, kind="ExternalInput")
    wk_d = nc.dram_tensor("wk", [D, CD], MMDT, kind="ExternalInput")
    wv_d = nc.dram_tensor("wv", [D, HPC * 65], MMDT, kind="ExternalInput")
    wf_d = nc.dram_tensor("wf", [CD, ATT], MMDT, kind="ExternalInput")
    mask_d = nc.dram_tensor("mask", [128, NJ], F32, kind="ExternalInput")
    bq_d = nc.dram_tensor("bqc", [128, 4], F32, kind="ExternalInput")
    bk_d = nc.dram_tensor("bkc", [128, 4], F32, kind="ExternalInput")
    y_d = nc.dram_tensor("y", [T, ATT], F32, kind="ExternalOutput")

    xq_r = xq_d[:, :].rearrange("(kc p) n -> p kc n", p=128)  # [128, 8, T]
    xv_r = xv_d[:, :].rearrange("(kc p) n -> p kc n", p=128)
    wq_r = wq_d[:, :].rearrange("(kc p) m -> p kc m", p=128)  # [128, 8, 512]
    wk_r = wk_d[:, :].rearrange("(kc p) m -> p kc m", p=128)
    wv_r = wv_d[:, :].rearrange("(kc p) m -> p kc m", p=128)  # [128, 8, 520]
    wf_r = wf_d[:, :].rearrange("(kc p) n -> p kc n", p=128)  # [128, 4, 1024]

    with tile.TileContext(nc) as tc:
        from contextlib import ExitStack
        _st = ExitStack()
        if loop_n > 1:
            _st.enter_context(tc.For_i(0, loop_n, 1))
        with _st, tc.tile_pool(name="persist", bufs=1) as persist:
            qT = persist.tile([128, 4, T], QKDT)
            kT = persist.tile([128, 4, TV], QKDT)
            v = persist.tile([128, NJ, HPC * 65], MMDT)
            mask = persist.tile([128, NJ], F32)
            bqc = persist.tile([128, 4], F32)
            bkc = persist.tile([128, 4], F32)
            ones = persist.tile([1, 64], MMDT)

            def set_ones(dst, src):
                nc.scalar.activation(out=dst, in_=src, func=AF.Identity,
                                     bias=1.0, scale=0.0)

            # ---------------- Phase A: projections ----------------
            with (
                tc.tile_pool(name="wpool", bufs=1) as wpool,
                tc.tile_pool(name="chunks", bufs=2 if NJ >= 15 else 3) as chunks,
                tc.tile_pool(name="ppq", bufs=4, space="PSUM") as ppq,
                tc.tile_pool(name="ppv", bufs=2, space="PSUM") as ppv,
            ):
                wq = wpool.tile([128, 8, CD], MMDT)
                wk = wpool.tile([128, 8, CD], MMDT)
                wv = wpool.tile([128, 8, HPC * 65], MMDT)
                for s in (range(T // SW) if ("A" in phases or "D" in phases)
                          else []):
                    sl = slice(s * SW, (s + 1) * SW)
                    if s < NSV:
                        xv_c = chunks.tile([128, 8, SW], MMDT, tag="xc")
                        nc.sync.dma_start(out=xv_c, in_=xv_r[:, :, sl])
                        if s == 0:
                            # wk/wv split per contraction chunk so the first
                            # projection matmuls start as soon as chunk 0 lands
                            for kc in range(8):
                                nc.sync.dma_start(out=wk[:, kc, :],
                                                  in_=wk_r[:, kc, :])
                            for kc in range(8):
                                nc.sync.dma_start(out=wv[:, kc, :],
                                                  in_=wv_r[:, kc, :])
                            nc.sync.dma_start(out=mask, in_=mask_d[:, :])
                            nc.sync.dma_start(out=bqc, in_=bq_d[:, :])
                            nc.sync.dma_start(out=bkc, in_=bk_d[:, :])
                            for kc in range(8):
                                nc.scalar.dma_start(out=wq[:, kc, :],
                                                    in_=wq_r[:, kc, :])
                        # kT columns for this span
                        for m in (range(4) if "A" in phases else []):
                            ps = ppq.tile([128, SW], F32, tag="qk")
                            for kc in range(8):
                                nc.tensor.matmul(
                                    ps[:, :],
                                    lhsT=wk[:, kc, m * 128:(m + 1) * 128],
                                    rhs=xv_c[:, kc, :],
                                    start=(kc == 0), stop=(kc == 7),
                                )
                            with nc.allow_low_precision(reason="qk store"):
                                nc.vector.tensor_scalar_add(
                                    kT[:, m, sl], ps[:, :], bkc[:, m:m + 1])
                        # v rows for this span (Tv tiles of 128)
                        for jt in (range(SW // 128) if "A" in phases else []):
                            j = s * (SW // 128) + jt
                            if j >= NJ:
                                continue
                            ps = ppv.tile([128, HPC * 65], F32, tag="v")
                            for kc in range(8):
                                nc.tensor.matmul(
                                    ps[:, 0:512],
                                    lhsT=xv_c[:, kc, jt * 128:(jt + 1) * 128],
                                    rhs=wv[:, kc, 0:512],
                                    start=(kc == 0), stop=(kc == 7),
                                )
                                nc.tensor.matmul(
                                    ps[:, 512:520],
                                    lhsT=xv_c[:, kc, jt * 128:(jt + 1) * 128],
                                    rhs=wv[:, kc, 512:520],
                                    start=(kc == 0), stop=(kc == 7),
                                )
                            nc.vector.tensor_copy(out=v[:, j, :], in_=ps[:, :])
                            vj = v[:, j, :].rearrange("p (h x) -> p h x", x=65)
                            set_ones(vj[:, :, 64:65], vj[:, :, 64:65])
                    # qT columns for this span
                    xq_c = chunks.tile([128, 8, SW], MMDT, tag="xc")
                    nc.scalar.dma_start(out=xq_c, in_=xq_r[:, :, sl])
                    for m in (range(4) if "A" in phases else []):
                        ps = ppq.tile([128, SW], F32, tag="qk")
                        for kc in range(8):
                            nc.tensor.matmul(
                                ps[:, :],
                                lhsT=wq[:, kc, m * 128:(m + 1) * 128],
                                rhs=xq_c[:, kc, :],
                                start=(kc == 0), stop=(kc == 7),
                            )
                        with nc.allow_low_precision(reason="qk store"):
                            nc.vector.tensor_scalar_add(
                                qT[:, m, sl], ps[:, :], bqc[:, m:m + 1])

            # ---------------- Phase B: attention ----------------
            with tc.tile_pool(name="bc_sbuf", bufs=1) as bcp:
                ctxT = bcp.tile([128, 4, T], MMDT)
                wf = bcp.tile([128, 4, ATT], MMDT)
                nc.scalar.dma_start(out=wf, in_=wf_r)
                if "Z" in phases:  # timing probe: fill ctxT without attention
                    for kc in range(4):
                        for cc in range(4):
                            set_ones(ctxT[:, kc, cc * 512:(cc + 1) * 512],
                                     ctxT[:, kc, cc * 512:(cc + 1) * 512])
                with (
                    tc.tile_pool(name="expp", bufs=exbufs) as expp,
                    tc.tile_pool(name="workp", bufs=4) as workp,
                    tc.tile_pool(name="yp", bufs=4) as yp,
                    tc.tile_pool(name="rsd", bufs=4, space="DRAM") as rsd,
                    tc.tile_pool(name="pe", bufs=ebufs, space="PSUM") as pe_pool,
                    tc.tile_pool(name="pcy", bufs=pcybufs, space="PSUM") as pcy,
                ):
                    def emit_c_unit(i, n):
                        y_ps = pcy.tile([128, 512], F32, tag="cy",
                                        name=f"y_{i}_{n}")
                        for kc in range(4):
                            nc.tensor.matmul(
                                y_ps[:, :],
                                lhsT=ctxT[:, kc, i * 128:(i + 1) * 128],
                                rhs=wf[:, kc, n * 512:(n + 1) * 512],
                                start=(kc == 0), stop=(kc == 3),
                            )
                        y_sb = yp.tile([128, 512], F32, tag="ysb")
                        nc.vector.tensor_copy(out=y_sb[:, :], in_=y_ps[:, :])
                        nc.scalar.dma_start(
                            out=y_d[i * 128:(i + 1) * 128,
                                    n * 512:(n + 1) * 512],
                            in_=y_sb[:, :])

                    # C units for block ib-1 are spliced into block ib's
                    # ACT-bound attention to fill PE idle slots
                    pending = []
                    for ib in range(4):  # Tq block of 512
                        ibs = slice(ib * 512, (ib + 1) * 512)
                        for hp in (range(4) if "B" in phases else []):
                            ctxA = pcy.tile([65, 512], F32, tag="cy")
                            ctxB = pcy.tile([65, 512], F32, tag="cy")
                            ctx_ps = (ctxA[:, :], ctxB[:, :])
                            for j in range(NJ):
                                e_ps = pe_pool.tile([128, 1024], F32, tag="e")
                                for hh in range(2):
                                    p0 = hh * 64
                                    nc.tensor.matmul(
                                        e_ps[:, hh * 512:(hh + 1) * 512],
                                        lhsT=kT[p0:p0 + 64, hp,
                                                j * 128:(j + 1) * 128],
                                        rhs=qT[p0:p0 + 64, hp, ibs],
                                        start=True, stop=True,
                                    )
                                ex = expp.tile([128, 1024], MMDT, tag="ex")
                                nc.scalar.activation(out=ex[:, :], in_=e_ps[:, :],
                                                     func=AF.Exp,
                                                     bias=mask[:, j:j + 1],
                                                     scale=1.0)
                                for hh in range(2):
                                    h = hp * 2 + hh
                                    nc.tensor.matmul(
                                        ctx_ps[hh],
                                        lhsT=v[:, j, h * 65:(h + 1) * 65],
                                        rhs=ex[:, hh * 512:(hh + 1) * 512],
                                        start=(j == 0), stop=(j == NJ - 1),
                                    )
                            for hh in range(2):
                                p0 = hh * 64
                                rs = workp.tile([1, 512], F32, tag="rs")
                                nc.vector.reciprocal(out=rs[:, :],
                                                     in_=ctx_ps[hh][64:65, :])
                                rs_dr = rsd.tile([1, 512], F32, tag="rsd")
                                nc.sync.dma_start(out=rs_dr[:, :], in_=rs[:, :])
                                bc_sb = workp.tile([64, 512], F32, tag="bcs")
                                nc.sync.dma_start(
                                    out=bc_sb[:, :],
                                    in_=rs_dr[0:1, :].partition_broadcast(64))
                                nc.vector.tensor_mul(
                                    ctxT[p0:p0 + 64, hp, ibs],
                                    ctx_ps[hh][0:64, :], bc_sb[:, :],
                                )
                            for _ in range(2):
                                if pending:
                                    emit_c_unit(*pending.pop(0))
                        while pending:
                            emit_c_unit(*pending.pop(0))
                        if "C" in phases:
                            pending = [(i, n) for i in range(ib * 4, ib * 4 + 4)
                                       for n in range(2)]
                    while pending:
                        emit_c_unit(*pending.pop(0))
    nc.compile()
    _cache[key] = nc
    return nc


def make_in_maps(query, value, value_lens, Wq, bq, Wk, bk, Wv, bv, Wf, bf,
                 mm_np=np.float32):
    query = np.ascontiguousarray(np.asarray(query, np.float32))
    value = np.ascontiguousarray(np.asarray(value, np.float32))
    value_lens = np.asarray(value_lens)
    Wq = np.asarray(Wq, np.float32)
    Wk = np.asarray(Wk, np.float32)
    Wv = np.asarray(Wv, np.float32)
    Wf = np.asarray(Wf, np.float32)
    bq = np.asarray(bq, np.float32)
    bk = np.asarray(bk, np.float32)

    scale = 1.0 / np.sqrt(np.float32(DH))
    effL = [int(l) if l > 0 else T for l in value_lens]
    NJ = max(1, int(np.ceil(max(effL) / 128)))

    in_maps = []
    for c in range(NCORES):
        b, hg = c // 2, c % 2
        L = int(value_lens[b])
        cs = slice(hg * CD, (hg + 1) * CD)
        xq = query[b].T.copy()
        if L == 0:
            xq = np.zeros_like(xq)
        xv = value[b].T.copy()
        wq = (Wq[:, cs] * scale).copy()
        wk = Wk[:, cs].copy()
        wv = np.zeros((D, HPC * 65), np.float32)
        for h in range(HPC):
            wv[:, h * 65:h * 65 + 64] = Wv[:, hg * CD + h * 64:hg * CD + (h + 1) * 64]
        wf = Wf[cs, :].copy()
        mask = np.zeros((128, NJ), np.float32)
        if L > 0:
            idx = np.arange(NJ * 128).reshape(NJ, 128).T  # [128, NJ]
            mask[idx >= L] = -LARGE
        bqc = (bq[cs] * scale).reshape(4, 128).T.copy()
        bkc = bk[cs].reshape(4, 128).T.copy()
        in_maps.append({
            "xq": xq.astype(mm_np), "xv": xv.astype(mm_np),
            "wq": wq.astype(mm_np), "wk": wk.astype(mm_np),
            "wv": wv.astype(mm_np), "wf": wf.astype(mm_np),
            "mask": mask, "bqc": bqc, "bkc": bkc,
        })
    return in_maps, NJ


def assemble(results, Wv, bv, Wf, bf):
    Wv = np.asarray(Wv, np.float32)
    bv = np.asarray(bv, np.float32)
    Wf = np.asarray(Wf, np.float32)
    bf = np.asarray(bf, np.float32)
    out = np.empty((B, T, ATT), np.float32)
    const = (bv @ Wf + bf).astype(np.float32)
    for b in range(B):
        out[b] = results[2 * b]["y"] + results[2 * b + 1]["y"] + const
    return out


def kernel(query, value, value_lens, Wq, bq, Wk, bk, Wv, bv, Wf, bf):
    in_maps, NJ = make_in_maps(query, value, value_lens, Wq, bq, Wk, bk,
                               Wv, bv, Wf, bf)
    nc = build_nc(NJ)
    res = run_bass_kernel_spmd(nc, in_maps, list(range(NCORES)))
    return assemble(res.results, Wv, bv, Wf, bf)



# revision 3
# speedup vs baseline: 1.1217x; 1.1217x over previous
"""Trainium2 Bass kernel for nn_DotAttention (B=4, Tq=Tv=2048, D=1024, 16 heads).

Sharding: core c -> (batch b = c//2, head-group hg = c%2 of 8 heads).
Each core computes q/k/v projections for its 512 att-dim slice, masked
softmax attention in transposed-energy layout, and a partial final
projection with its 512-row slice of Wf. Host sums the two partials per
batch and adds the bias constant (bv @ Wf + bf, exact because attention
weights sum to 1).

Layouts (SBUF is [128 partitions, free]):
  qT/kT  [128, 4, T]   partition+chunk = att-dim slice c, free = time
  v      [128, NJ, 520] partition = Tv tile, per head 65 cols (64 v + ones)
  energy^T in PSUM [128(Tv), 2*512] both heads of a pair side by side
  ctx^T  in PSUM [65, 512] per head; row 64 = softmax denominator (ones col)

All matmuls run in float32r (TF32-like, full PE rate at N>=256).
The program is specialized on NJ = ceil(max(value_lens)/128): fully
masked Tv chunks beyond that are skipped; per-core masking is handled by
a per-partition additive bias (-1e30) on the exp activation.
"""

import sys

sys.path.insert(0, "/opt/trn_rl_repo")

import numpy as np

import concourse.bacc as bacc
import concourse.tile as tile
import concourse.mybir as mybir
from concourse.bass_utils import run_bass_kernel_spmd

F32 = mybir.dt.float32
F32R = mybir.dt.float32r
BF16 = mybir.dt.bfloat16
F16 = mybir.dt.float16
MMDT = BF16
AF = mybir.ActivationFunctionType

B, T, D, ATT = 4, 2048, 1024, 1024
NH, DH = 16, 64
HPC = 8  # heads per core
CD = 512  # att-dim slice per core
NCORES = 8
LARGE = 1e30
SW = 512  # time-span width per streamed input chunk

_cache = {}


def build_nc(NJ, phases="ABC", loop_n=1, mmdt=None, splice=True,
             ebufs=2, pcybufs=4, exbufs=3, ehalf=False, pebc=False,
             qkf16=False):
    global MMDT
    if mmdt is not None:
        MMDT = mmdt
    key = (NJ, phases, loop_n, str(MMDT), splice, ebufs, pcybufs, exbufs, ehalf, pebc, qkf16)
    QKDT = F16 if qkf16 else MMDT
    if key in _cache:
        return _cache[key]
    NSV = (NJ * 128 + SW - 1) // SW  # spans of Tv needed for k/v
    TV = NSV * SW  # padded Tv extent materialized for kT
    nc = bacc.Bacc("TRN2", target_bir_lowering=False, debug=False, num_devices=NCORES)

    xq_d = nc.dram_tensor("xq", [D, T], MMDT, kind="ExternalInput")  # query[b].T
    xv_d = nc.dram_tensor("xv", [D, T], MMDT, kind="ExternalInput")  # value[b].T
    wq_d = nc.dram_tensor("wq", [D, CD], MMDT, kind="ExternalInput")
    wk_d = nc.dram_tensor("wk", [D, CD], MMDT, kind="ExternalInput")
    wv_d = nc.dram_tensor("wv", [D, HPC * 65], MMDT, kind="ExternalInput")
    wf_d = nc.dram_tensor("wf", [CD, ATT], MMDT, kind="ExternalInput")
    mask_d = nc.dram_tensor("mask", [128, NJ], F32, kind="ExternalInput")
    bq_d = nc.dram_tensor("bqc", [128, 4], F32, kind="ExternalInput")
    bk_d = nc.dram_tensor("bkc", [128, 4], F32, kind="ExternalInput")
    y_d = nc.dram_tensor("y", [T, ATT], F32, kind="ExternalOutput")

    xq_r = xq_d[:, :].rearrange("(kc p) n -> p kc n", p=128)  # [128, 8, T]
    xv_r = xv_d[:, :].rearrange("(kc p) n -> p kc n", p=128)
    wq_r = wq_d[:, :].rearrange("(kc p) m -> p kc m", p=128)  # [128, 8, 512]
    wk_r = wk_d[:, :].rearrange("(kc p) m -> p kc m", p=128)
    wv_r = wv_d[:, :].rearrange("(kc p) m -> p kc m", p=128)  # [128, 8, 520]
    wf_r = wf_d[:, :].rearrange("(kc p) n -> p kc n", p=128)  # [128, 4, 1024]

    with tile.TileContext(nc) as tc:
        from contextlib import ExitStack
        _st = ExitStack()
        if loop_n > 1:
            _st.enter_context(tc.For_i(0, loop_n, 1))
        with _st, tc.tile_pool(name="persist", bufs=1) as persist:
            qT = persist.tile([128, 4, T], QKDT)
            kT = persist.tile([128, 4, TV], QKDT)
            v = persist.tile([128, NJ, HPC * 65], MMDT)
            mask = persist.tile([128, NJ], F32)
            bqc = persist.tile([128, 4], F32)
            bkc = persist.tile([128, 4], F32)
            ones = persist.tile([1, 64], MMDT)

            def set_ones(dst, src):
                nc.scalar.activation(out=dst, in_=src, func=AF.Identity,
                                     bias=1.0, scale=0.0)

            # ---------------- Phase A: projections ----------------
            with (
                tc.tile_pool(name="wpool", bufs=1) as wpool,
                tc.tile_pool(name="chunks", bufs=2 if NJ >= 15 else 3) as chunks,
                tc.tile_pool(name="ppq", bufs=4, space="PSUM") as ppq,
                tc.tile_pool(name="ppv", bufs=2, space="PSUM") as ppv,
            ):
                wq = wpool.tile([128, 8, CD], MMDT)
                wk = wpool.tile([128, 8, CD], MMDT)
                wv = wpool.tile([128, 8, HPC * 65], MMDT)
                for s in (range(T // SW) if ("A" in phases or "D" in phases)
                          else []):
                    sl = slice(s * SW, (s + 1) * SW)
                    if s < NSV:
                        xv_c = chunks.tile([128, 8, SW], MMDT, tag="xc")
                        nc.sync.dma_start(out=xv_c, in_=xv_r[:, :, sl])
                        if s == 0:
                            # wk/wv split per contraction chunk so the first
                            # projection matmuls start as soon as chunk 0 lands
                            for kc in range(8):
                                nc.sync.dma_start(out=wk[:, kc, :],
                                                  in_=wk_r[:, kc, :])
                            for kc in range(8):
                                nc.sync.dma_start(out=wv[:, kc, :],
                                                  in_=wv_r[:, kc, :])
                            nc.sync.dma_start(out=mask, in_=mask_d[:, :])
                            nc.sync.dma_start(out=bqc, in_=bq_d[:, :])
                            nc.sync.dma_start(out=bkc, in_=bk_d[:, :])
                            for kc in range(8):
                                nc.scalar.dma_start(out=wq[:, kc, :],
                                                    in_=wq_r[:, kc, :])
                        # kT columns for this span
                        for m in (range(4) if "A" in phases else []):
                            ps = ppq.tile([128, SW], F32, tag="qk")
                            for kc in range(8):
                                nc.tensor.matmul(
                                    ps[:, :],
                                    lhsT=wk[:, kc, m * 128:(m + 1) * 128],
                                    rhs=xv_c[:, kc, :],
                                    start=(kc == 0), stop=(kc == 7),
                                )
                            with nc.allow_low_precision(reason="qk store"):
                                nc.vector.tensor_scalar_add(
                                    kT[:, m, sl], ps[:, :], bkc[:, m:m + 1])
                        # v rows for this span (Tv tiles of 128)
                        for jt in (range(SW // 128) if "A" in phases else []):
                            j = s * (SW // 128) + jt
                            if j >= NJ:
                                continue
                            ps = ppv.tile([128, HPC * 65], F32, tag="v")
                            for kc in range(8):
                                nc.tensor.matmul(
                                    ps[:, 0:512],
                                    lhsT=xv_c[:, kc, jt * 128:(jt + 1) * 128],
                                    rhs=wv[:, kc, 0:512],
                                    start=(kc == 0), stop=(kc == 7),
                                )
                                nc.tensor.matmul(
                                    ps[:, 512:520],
                                    lhsT=xv_c[:, kc, jt * 128:(jt + 1) * 128],
                                    rhs=wv[:, kc, 512:520],
                                    start=(kc == 0), stop=(kc == 7),
                                )
                            nc.vector.tensor_copy(out=v[:, j, :], in_=ps[:, :])
                            vj = v[:, j, :].rearrange("p (h x) -> p h x", x=65)
                            set_ones(vj[:, :, 64:65], vj[:, :, 64:65])
                    # qT columns for this span
                    xq_c = chunks.tile([128, 8, SW], MMDT, tag="xc")
                    nc.scalar.dma_start(out=xq_c, in_=xq_r[:, :, sl])
                    for m in (range(4) if "A" in phases else []):
                        ps = ppq.tile([128, SW], F32, tag="qk")
                        for kc in range(8):
                            nc.tensor.matmul(
                                ps[:, :],
                                lhsT=wq[:, kc, m * 128:(m + 1) * 128],
                                rhs=xq_c[:, kc, :],
                                start=(kc == 0), stop=(kc == 7),
                            )
                        with nc.allow_low_precision(reason="qk store"):
                            nc.vector.tensor_scalar_add(
                                qT[:, m, sl], ps[:, :], bqc[:, m:m + 1])

            # ---------------- Phase B: attention ----------------
            with tc.tile_pool(name="bc_sbuf", bufs=1) as bcp:
                ctxT = bcp.tile([128, 4, T], MMDT)
                wf = bcp.tile([128, 4, ATT], MMDT)
                nc.scalar.dma_start(out=wf, in_=wf_r)
                if "Z" in phases:  # timing probe: fill ctxT without attention
                    for kc in range(4):
                        for cc in range(4):
                            set_ones(ctxT[:, kc, cc * 512:(cc + 1) * 512],
                                     ctxT[:, kc, cc * 512:(cc + 1) * 512])
                with (
                    tc.tile_pool(name="expp", bufs=exbufs) as expp,
                    tc.tile_pool(name="workp", bufs=4) as workp,
                    tc.tile_pool(name="yp", bufs=4) as yp,
                    tc.tile_pool(name="rsd", bufs=4, space="DRAM") as rsd,
                    tc.tile_pool(name="pe", bufs=ebufs, space="PSUM") as pe_pool,
                    tc.tile_pool(name="pcy", bufs=pcybufs, space="PSUM") as pcy,
                ):
                    def emit_c_unit(i, n):
                        y_ps = pcy.tile([128, 512], F32, tag="cy",
                                        name=f"y_{i}_{n}")
                        for kc in range(4):
                            nc.tensor.matmul(
                                y_ps[:, :],
                                lhsT=ctxT[:, kc, i * 128:(i + 1) * 128],
                                rhs=wf[:, kc, n * 512:(n + 1) * 512],
                                start=(kc == 0), stop=(kc == 3),
                            )
                        y_sb = yp.tile([128, 512], F32, tag="ysb")
                        nc.vector.tensor_copy(out=y_sb[:, :], in_=y_ps[:, :])
                        nc.scalar.dma_start(
                            out=y_d[i * 128:(i + 1) * 128,
                                    n * 512:(n + 1) * 512],
                            in_=y_sb[:, :])

                    # C units for block ib-1 are spliced into block ib's
                    # ACT-bound attention to fill PE idle slots
                    pending = []
                    for ib in range(4):  # Tq block of 512
                        ibs = slice(ib * 512, (ib + 1) * 512)
                        for hp in (range(4) if "B" in phases else []):
                            ctxA = pcy.tile([65, 512], F32, tag="cy")
                            ctxB = pcy.tile([65, 512], F32, tag="cy")
                            ctx_ps = (ctxA[:, :], ctxB[:, :])
                            for j in range(NJ):
                                e_ps = pe_pool.tile([128, 1024], F32, tag="e")
                                for hh in range(2):
                                    p0 = hh * 64
                                    nc.tensor.matmul(
                                        e_ps[:, hh * 512:(hh + 1) * 512],
                                        lhsT=kT[p0:p0 + 64, hp,
                                                j * 128:(j + 1) * 128],
                                        rhs=qT[p0:p0 + 64, hp, ibs],
                                        start=True, stop=True,
                                    )
                                ex = expp.tile([128, 1024], MMDT, tag="ex")
                                nc.scalar.activation(out=ex[:, :], in_=e_ps[:, :],
                                                     func=AF.Exp,
                                                     bias=mask[:, j:j + 1],
                                                     scale=1.0)
                                for hh in range(2):
                                    h = hp * 2 + hh
                                    nc.tensor.matmul(
                                        ctx_ps[hh],
                                        lhsT=v[:, j, h * 65:(h + 1) * 65],
                                        rhs=ex[:, hh * 512:(hh + 1) * 512],
                                        start=(j == 0), stop=(j == NJ - 1),
                                    )
                            for hh in range(2):
                                p0 = hh * 64
                                rs = workp.tile([1, 512], F32, tag="rs")
                                nc.vector.reciprocal(out=rs[:, :],
                                                     in_=ctx_ps[hh][64:65, :])
                                rs_dr = rsd.tile([1, 512], F32, tag="rsd")
                                nc.sync.dma_start(out=rs_dr[:, :], in_=rs[:, :])
                                bc_sb = workp.tile([64, 512], F32, tag="bcs")
                                nc.sync.dma_start(
                                    out=bc_sb[:, :],
                                    in_=rs_dr[0:1, :].partition_broadcast(64))
                                nc.vector.tensor_mul(
                                    ctxT[p0:p0 + 64, hp, ibs],
                                    ctx_ps[hh][0:64, :], bc_sb[:, :],
                                )
                            for _ in range(2):
                                if pending:
                                    emit_c_unit(*pending.pop(0))
                        while pending:
                            emit_c_unit(*pending.pop(0))
                        if "C" in phases:
                            pending = [(i, n) for i in range(ib * 4, ib * 4 + 4)
                                       for n in range(2)]
                    while pending:
                        emit_c_unit(*pending.pop(0))
    nc.compile()
    _cache[key] = nc
    return nc


import ml_dtypes

def make_in_maps(query, value, value_lens, Wq, bq, Wk, bk, Wv, bv, Wf, bf,
                 mm_np=ml_dtypes.bfloat16):
    query = np.ascontiguousarray(np.asarray(query, np.float32))
    value = np.ascontiguousarray(np.asarray(value, np.float32))
    value_lens = np.asarray(value_lens)
    Wq = np.asarray(Wq, np.float32)
    Wk = np.asarray(Wk, np.float32)
    Wv = np.asarray(Wv, np.float32)
    Wf = np.asarray(Wf, np.float32)
    bq = np.asarray(bq, np.float32)
    bk = np.asarray(bk, np.float32)

    scale = 1.0 / np.sqrt(np.float32(DH))
    effL = [int(l) if l > 0 else T for l in value_lens]
    NJ = max(1, int(np.ceil(max(effL) / 128)))

    in_maps = []
    for c in range(NCORES):
        b, hg = c // 2, c % 2
        L = int(value_lens[b])
        cs = slice(hg * CD, (hg + 1) * CD)
        xq = query[b].T.copy()
        if L == 0:
            xq = np.zeros_like(xq)
        xv = value[b].T.copy()
        wq = (Wq[:, cs] * scale).copy()
        wk = Wk[:, cs].copy()
        wv = np.zeros((D, HPC * 65), np.float32)
        for h in range(HPC):
            wv[:, h * 65:h * 65 + 64] = Wv[:, hg * CD + h * 64:hg * CD + (h + 1) * 64]
        wf = Wf[cs, :].copy()
        mask = np.zeros((128, NJ), np.float32)
        if L > 0:
            idx = np.arange(NJ * 128).reshape(NJ, 128).T  # [128, NJ]
            mask[idx >= L] = -LARGE
        bqc = (bq[cs] * scale).reshape(4, 128).T.copy()
        bkc = bk[cs].reshape(4, 128).T.copy()
        in_maps.append({
            "xq": xq.astype(mm_np), "xv": xv.astype(mm_np),
            "wq": wq.astype(mm_np), "wk": wk.astype(mm_np),
            "wv": wv.astype(mm_np), "wf": wf.astype(mm_np),
            "mask": mask, "bqc": bqc, "bkc": bkc,
        })
    return in_maps, NJ


def assemble(results, Wv, bv, Wf, bf):
    Wv = np.asarray(Wv, np.float32)
    bv = np.asarray(bv, np.float32)
    Wf = np.asarray(Wf, np.float32)
    bf = np.asarray(bf, np.float32)
    out = np.empty((B, T, ATT), np.float32)
    const = (bv @ Wf + bf).astype(np.float32)
    for b in range(B):
        out[b] = results[2 * b]["y"] + results[2 * b + 1]["y"] + const
    return out


def kernel(query, value, value_lens, Wq, bq, Wk, bk, Wv, bv, Wf, bf):
    in_maps, NJ = make_in_maps(query, value, value_lens, Wq, bq, Wk, bk,
                               Wv, bv, Wf, bf)
    nc = build_nc(NJ)
    res = run_bass_kernel_spmd(nc, in_maps, list(range(NCORES)))
    return assemble(res.results, Wv, bv, Wf, bf)

